# revision 19
# baseline (speedup 1.0000x reference)
"""Trainium2 Bass kernel: single-head causal attention.

B=4, T=4096, E=512, H=64, fp32 in/out.

Sharding: 2 cores per batch sample. Each core computes partial softmax
(numerator and denominator) for ALL 4096 queries of its sample over HALF
the keys: core 2b takes even 128-key-strips, core 2b+1 odd strips. The
host combines partials: out = (num0+num1)/(den0+den1).

Token rotation: the host rotates every 256-token block by 128*rho so
each core's keys are the first 128 tokens of every 256-block (identical
SPMD program on all cores). Host un-permutes output columns; the causal
masks carry the rotation.

Device kernel per core (bf16 matmul operands, fp32 PSUM accumulate):
  - ROW-TILED scores: the two strips of a pair run concurrently in the
    PE array (strip A rows 0:64, strip B rows 64:128 via operand base
    partitions). Q projection duplicated to both partition halves
    ([Wq|Wq] stationary). The kv projection is PARITY-SPLIT so odd
    strips' K^T lands directly at partitions 64:128: even-strip token
    columns use the [Wk|Wv] stationary, odd-strip columns [Wv|Wk]
    (so odd strips' V^T is at rows 0:64 instead).
  - V^T -> V (natural [k,h]) via PE transpose (4 per kv chunk into one
    bf16 PSUM tile) + one strided DVE copy per chunk; a ones column is
    packed after each V strip so the PV matmul (M=65) also produces the
    softmax denominator (partition-dim reduction on the PE).
  - exp on the scalar engine with fused 1/sqrt(H) scale; no max
    subtraction (scores bounded; fp32 exp cannot overflow here).
  - Diagonal pair of each chunk: the fully-masked half of the odd strip
    is skipped (scores/exp/PV restricted to query cols 256:512); the
    exp covers the contiguous [256:1024] window of the pair tile.
  - Causal mask applied multiplicatively after exp on the vector
    engine (256-col windows per diagonal strip).
  - Emission is software-pipelined: chunk c's first scores pair goes
    out before the c+1 projections, and scores/exp of pair j+1 precede
    PV of pair j, so the in-order PE never waits on exp or on input
    DMAs.
  - PE warm-up matmuls run during the input DMA window so real matmuls
    start at the full 2.4 GHz clock (HAM un-throttled).
"""

import functools

import numpy as np
import ml_dtypes

B, T, E, H = 4, 4096, 512, 64
NCORES = 8
NCHUNK = 8  # 512-query chunks per sample
CHUNK = T // NCHUNK  # 512
NSTRIP = 16  # local 128-key strips per core (half of T/128)
VSTRIDE = 80  # per-strip stride in the packed V tile

bf16 = ml_dtypes.bfloat16


@functools.lru_cache(maxsize=1)
def _build():
    import concourse.mybir as mybir
    from concourse import bacc
    from concourse.masks import make_identity
    import concourse.tile as tile

    dt_bf = mybir.dt.bfloat16
    dt_f32 = mybir.dt.float32

    nc = bacc.Bacc("TRN2", target_bir_lowering=False, num_devices=NCORES)

    # x^T, rotated, (quarter, e-strip)-blocked:
    # [4 quarters, 128, 4 e-strips, 1024 tokens]
    xt = nc.dram_tensor("xt", [4, 128, 4, T // 4], dt_bf, kind="ExternalInput")
    # [Wq | Wq] per e-strip (duplicated for row-tiled scores)
    wq = nc.dram_tensor("wq", [128, 4 * 128], dt_bf, kind="ExternalInput")
    # per e-strip: [Wk|Wv] (even-strip cols) then [Wv|Wk] (odd-strip cols)
    wkv = nc.dram_tensor("wkv", [128, 4 * 256], dt_bf, kind="ExternalInput")
    bias_q = nc.dram_tensor("bias_q", [128, 1], dt_f32, kind="ExternalInput")
    # [bk|bv] and [bv|bk]
    bias_kv = nc.dram_tensor("bias_kv", [128, 2], dt_f32, kind="ExternalInput")
    masks = nc.dram_tensor("masks", [128, CHUNK], dt_bf, kind="ExternalInput")
    out_d = nc.dram_tensor("out", [H + 1, T], dt_f32, kind="ExternalOutput")

    AF = mybir.ActivationFunctionType

    with tile.TileContext(nc) as tc:
        with (
            tc.tile_pool(name="const", bufs=1) as cpool,
            tc.tile_pool(name="xt_pool", bufs=1) as xpool,
            tc.tile_pool(name="q_pool", bufs=NCHUNK) as qpool,
            tc.tile_pool(name="kv_pool", bufs=4) as kvpool,
            tc.tile_pool(name="v_pool", bufs=1) as vpool,
            tc.tile_pool(name="p_pool", bufs=3) as ppool,
            tc.tile_pool(name="o_pool", bufs=2) as opool,
            tc.tile_pool(name="ps_proj", bufs=2, space="PSUM") as pspr_pool,
            tc.tile_pool(name="ps_s", bufs=2, space="PSUM") as pss_pool,
            tc.tile_pool(name="ps_o", bufs=2, space="PSUM") as pso_pool,
        ):
            # ---- input DMAs. The critical path (wkv + quarter-0
            # e-strips) leads on the sync queue, followed by the other
            # quarters in order; the scalar queue carries only the small
            # weight/bias/mask transfers so nothing competes with the
            # quarter-0 stream for SDMA bandwidth ----
            xt_sb = xpool.tile([128, 4 * T], dt_bf)
            wkv_sb = cpool.tile([128, 4 * 256], dt_bf)
            nc.scalar.dma_start(wkv_sb, wkv.ap())
            for es in range(4):
                nc.sync.dma_start(
                    xt_sb[:, es * 1024 : (es + 1) * 1024], xt.ap()[0][:, es, :]
                )
            wq_sb = cpool.tile([128, 4 * 128], dt_bf)
            nc.scalar.dma_start(wq_sb, wq.ap())
            bkv_sb = cpool.tile([128, 2], dt_f32)
            nc.scalar.dma_start(bkv_sb, bias_kv.ap())
            bq_sb = cpool.tile([128, 1], dt_f32)
            nc.scalar.dma_start(bq_sb, bias_q.ap())
            masks_sb = cpool.tile([128, CHUNK], dt_bf)
            nc.scalar.dma_start(masks_sb, masks.ap())
            for qd in range(1, 4):
                nc.sync.dma_start(
                    xt_sb[:, qd * T : qd * T + T // 2],
                    xt.ap()[qd][:, 0:2, :].rearrange("p a t -> p (a t)"),
                )
                nc.sync.dma_start(
                    xt_sb[:, qd * T + T // 2 : (qd + 1) * T],
                    xt.ap()[qd][:, 2:4, :].rearrange("p a t -> p (a t)"),
                )
            ident = cpool.tile([128, 128], dt_bf)
            make_identity(nc, ident)
            junk_sb = cpool.tile([128, CHUNK], dt_bf)
            nc.vector.memset(junk_sb, 0.0)

            # ---- PE warm-up: keep the PE busy through the HAM window
            # until the first e-strips land (results never read) ----
            for i in range(4):
                ps_w = pss_pool.tile([128, 2 * CHUNK], dt_f32, tag="pss")
                for r in range(2):
                    nc.tensor.matmul(
                        ps_w[:, r * CHUNK : (r + 1) * CHUNK],
                        lhsT=junk_sb[:, 0:128],
                        rhs=junk_sb,
                        start=True,
                        stop=True,
                        skip_group_check=True,
                    )

            # packed V (natural [k,h] layout + ones column for denominator)
            v_nat = vpool.tile([128, NSTRIP * VSTRIDE], dt_bf)
            v3 = v_nat.rearrange("p (s c) -> p s c", c=VSTRIDE)
            nc.vector.memset(v3[:, :, 64:65], 1.0)

            def xt_block(qd, es):
                off = (qd * 4 + es) * 1024
                return xt_sb[:, off : off + 1024]

            scale = 1.0 / float(np.sqrt(H))
            kv_tiles = []
            q_tiles = []

            def kv_proj(ckv):
                # kv tile layout: cols [0:256] = even strips (0,2) with
                # [K^T;V^T] rows, cols [256:512] = odd strips (1,3) with
                # [V^T;K^T] rows (so odd K^T sits at partitions 64:128).
                # The two parity accumulation chains must live in
                # DIFFERENT PSUM banks: a start=True matmul clears the
                # has_written bits of its whole bank, so a second chain's
                # start would break the first chain's accumulation.
                ps_e = pspr_pool.tile([128, 256], dt_f32, tag="proj")
                ps_od = pss_pool.tile([128, 2 * CHUNK], dt_f32, tag="pss")
                for es in range(4):
                    key_rhs = xt_block(ckv, es).rearrange(
                        "p (a two b) -> p a two b", two=2, b=128
                    )[:, :, 0, :]
                    nc.tensor.matmul(
                        ps_e,
                        lhsT=wkv_sb[:, es * 256 : es * 256 + 128],
                        rhs=key_rhs[:, 0::2, :],
                        start=(es == 0),
                        stop=(es == 3),
                    )
                    nc.tensor.matmul(
                        ps_od[:, 0:256],
                        lhsT=wkv_sb[:, es * 256 + 128 : es * 256 + 256],
                        rhs=key_rhs[:, 1::2, :],
                        start=(es == 0),
                        stop=(es == 3),
                    )
                kv_sb = kvpool.tile([128, CHUNK], dt_bf, tag="kv")
                nc.vector.tensor_scalar_add(
                    kv_sb[:, 0:256], ps_e, bkv_sb[:, 0:1]
                )
                nc.vector.tensor_scalar_add(
                    kv_sb[:, 256:512], ps_od[:, 0:256], bkv_sb[:, 1:2]
                )
                kv_tiles.append(kv_sb)

            def v_transpose(ckv):
                # V^T -> natural V strips via PE transpose. Even strips'
                # V^T is at rows 64:128 (-> V cols 64:128 of the
                # transpose), odd strips' at rows 0:64 (-> V cols 0:64).
                # Transposed block order: [s0, s2, s1, s3].
                kv_sb = kv_tiles[ckv]
                ps_tr = pspr_pool.tile([128, CHUNK], dt_bf, tag="proj")
                for j in range(4):
                    nc.tensor.transpose(
                        ps_tr[:, j * 128 : (j + 1) * 128],
                        kv_sb[:, j * 128 : (j + 1) * 128],
                        ident,
                    )
                sl = v3[:, 4 * ckv : 4 * ckv + 4, 0:64]
                ev = ps_tr[:, 64:320].rearrange("p (s c) -> p s c", c=128)
                od = ps_tr[:, 256:512].rearrange("p (s c) -> p s c", c=128)
                # even strips s0, s2 -> v slots 0, 2; odd s1, s3 -> 1, 3
                nc.vector.tensor_copy(sl[:, 0::2, :], ev[:, :, 0:64])
                nc.vector.tensor_copy(sl[:, 1::2, :], od[:, :, 0:64])

            def q_proj(c):
                ps_q = pspr_pool.tile([128, CHUNK], dt_f32, tag="proj")
                for es in range(4):
                    nc.tensor.matmul(
                        ps_q,
                        lhsT=wq_sb[:, es * 128 : (es + 1) * 128],
                        rhs=xt_block(c // 2, es)[
                            :, (c % 2) * CHUNK : (c % 2) * CHUNK + CHUNK
                        ],
                        start=(es == 0),
                        stop=(es == 3),
                    )
                q_sb = qpool.tile([128, CHUNK], dt_bf, tag="q")
                nc.vector.tensor_scalar_add(q_sb, ps_q, bq_sb)
                q_tiles.append(q_sb)

            def k_lo(l):
                # even strip l: K^T at partitions 0:64, even half cols
                o = ((l % 4) // 2) * 128
                return kv_tiles[l // 4][0:64, o : o + 128]

            def k_hi(l):
                # odd strip l: K^T at partitions 64:128, odd half cols
                o = 256 + ((l % 4) // 2) * 128
                return kv_tiles[l // 4][64:128, o : o + 128]

            kv_proj(0)
            q_proj(0)
            for c in range(NCHUNK):
                ns = 2 * (c + 1)
                q_sb = q_tiles[c]
                ps_o = pso_pool.tile([H + 1, CHUNK], dt_f32, tag="pso")

                def scores_exp(j):
                    le, lo = 2 * j, 2 * j + 1
                    diag = j == c
                    ps_s = pss_pool.tile([128, 2 * CHUNK], dt_f32, tag="pss")
                    p_sb = ppool.tile([128, 2 * CHUNK], dt_bf, tag="p")
                    if not diag:
                        nc.tensor.matmul(
                            ps_s[:, 0:CHUNK],
                            lhsT=k_lo(le),
                            rhs=q_sb[0:64, :],
                            start=True,
                            stop=True,
                        )
                        nc.tensor.matmul(
                            ps_s[:, CHUNK : 2 * CHUNK],
                            lhsT=k_hi(lo),
                            rhs=q_sb[64:128, :],
                            start=True,
                            stop=True,
                        )
                        nc.scalar.activation(p_sb, ps_s, AF.Exp, scale=scale)
                    else:
                        # diagonal pair: odd strip only needs query cols
                        # 256:512; layout [odd | even] so one exp covers
                        # the contiguous [256:1024] window
                        nc.tensor.matmul(
                            ps_s[:, 256:CHUNK],
                            lhsT=k_hi(lo),
                            rhs=q_sb[64:128, 256:CHUNK],
                            start=True,
                            stop=True,
                        )
                        nc.tensor.matmul(
                            ps_s[:, CHUNK : 2 * CHUNK],
                            lhsT=k_lo(le),
                            rhs=q_sb[0:64, :],
                            start=True,
                            stop=True,
                        )
                        nc.scalar.activation(
                            p_sb[:, 256 : 2 * CHUNK],
                            ps_s[:, 256 : 2 * CHUNK],
                            AF.Exp,
                            scale=scale,
                        )
                        nc.vector.tensor_mul(
                            p_sb[:, 256:CHUNK],
                            p_sb[:, 256:CHUNK],
                            masks_sb[:, 0:256],
                        )
                        nc.vector.tensor_mul(
                            p_sb[:, CHUNK : CHUNK + 256],
                            p_sb[:, CHUNK : CHUNK + 256],
                            masks_sb[:, 256:512],
                        )
                    return p_sb

                def pv(j, p_sb):
                    le, lo = 2 * j, 2 * j + 1
                    diag = j == c
                    if not diag:
                        nc.tensor.matmul(
                            ps_o,
                            lhsT=v_nat[:, le * VSTRIDE : le * VSTRIDE + 65],
                            rhs=p_sb[:, 0:CHUNK],
                            start=(le == 0),
                            stop=False,
                        )
                        nc.tensor.matmul(
                            ps_o,
                            lhsT=v_nat[:, lo * VSTRIDE : lo * VSTRIDE + 65],
                            rhs=p_sb[:, CHUNK : 2 * CHUNK],
                            start=False,
                            stop=(lo == ns - 1),
                        )
                    else:
                        nc.tensor.matmul(
                            ps_o,
                            lhsT=v_nat[:, le * VSTRIDE : le * VSTRIDE + 65],
                            rhs=p_sb[:, CHUNK : 2 * CHUNK],
                            start=(le == 0),
                            stop=False,
                        )
                        nc.tensor.matmul(
                            ps_o[:, 256:CHUNK],
                            lhsT=v_nat[:, lo * VSTRIDE : lo * VSTRIDE + 65],
                            rhs=p_sb[:, 256:CHUNK],
                            start=False,
                            stop=(lo == ns - 1),
                            skip_group_check=True,
                        )

                # chunk c's first scores pair goes out BEFORE the c+1
                # projections (which may wait on later input DMAs)
                prev = scores_exp(0)
                if c % 2 == 0:
                    v_transpose(c // 2)
                if c + 1 < NCHUNK:
                    if (c + 1) % 2 == 0:
                        kv_proj((c + 1) // 2)
                    q_proj(c + 1)
                for j in range(1, c + 1):
                    cur = scores_exp(j)
                    pv(j - 1, prev)
                    prev = cur
                pv(c, prev)

                o_sb = opool.tile([H + 1, CHUNK], dt_f32, tag="o")
                nc.vector.tensor_copy(o_sb, ps_o)
                nc.sync.dma_start(
                    out_d.ap()[:, c * CHUNK : (c + 1) * CHUNK], o_sb
                )

    nc.compile()
    return nc


def _perm(rho):
    """Rotated-order permutation: rotated position i holds original token
    perm[i]. Involutive (half swap within each 256-block)."""
    i = np.arange(T)
    return (i // 256) * 256 + ((i % 256) + 128 * rho) % 256


def _make_in_maps(x, Wq, bq, Wk, bk, Wv, bv):
    wq_r = Wq.reshape(4, 128, 64)
    wq_pack = np.ascontiguousarray(
        np.concatenate([wq_r, wq_r], axis=2).transpose(1, 0, 2).reshape(128, 512)
    ).astype(bf16)
    wk_r = Wk.reshape(4, 128, 64)
    wv_r = Wv.reshape(4, 128, 64)
    # per e-strip: [Wk|Wv] then [Wv|Wk]
    wkv_pack = np.ascontiguousarray(
        np.concatenate([wk_r, wv_r, wv_r, wk_r], axis=2)
        .transpose(1, 0, 2)
        .reshape(128, 1024)
    ).astype(bf16)
    bias_q = np.ascontiguousarray(
        np.concatenate([bq, bq])[:, None]
    ).astype(np.float32)
    bias_kv = np.ascontiguousarray(
        np.stack([np.concatenate([bk, bv]), np.concatenate([bv, bk])], axis=1)
    ).astype(np.float32)

    kk = np.arange(128)[:, None]
    in_maps = []
    for b in range(B):
        xt_b = np.ascontiguousarray(x[b].T).astype(bf16).reshape(4, 128, T)
        for rho in range(2):
            perm = _perm(rho)
            xt_rot = xt_b[:, :, perm]  # rotated token order
            xt_in = np.ascontiguousarray(
                xt_rot.reshape(4, 128, 4, T // 4).transpose(2, 1, 0, 3)
            )
            v = perm[:CHUNK]
            m0 = (kk - v[None, :] <= -128 * rho).astype(bf16)
            m1 = (kk - v[None, :] <= -256 - 128 * rho).astype(bf16)
            # [odd-strip window cols 256:512 | even-strip cols 0:256]
            masks_np = np.ascontiguousarray(
                np.concatenate([m1[:, 256:512], m0[:, 0:256]], axis=1)
            )
            in_maps.append(
                {
                    "xt": xt_in,
                    "wq": wq_pack,
                    "wkv": wkv_pack,
                    "bias_q": bias_q,
                    "bias_kv": bias_kv,
                    "masks": masks_np,
                }
            )
    return in_maps


def _combine(results):
    out = np.empty((B, T, H), np.float32)
    p1 = _perm(1)
    for b in range(B):
        a0 = results[2 * b]["out"].astype(np.float64)
        a1 = results[2 * b + 1]["out"].astype(np.float64)
        a1 = a1[:, p1]  # un-rotate core-1 columns (involutive perm)
        num = a0[:H] + a1[:H]
        den = a0[H] + a1[H]
        out[b] = (num / den).T.astype(np.float32)
    return out


def _run(trace=False, **inputs):
    from concourse import bass_utils

    nc = _build()
    in_maps = _make_in_maps(
        np.asarray(inputs["x"], np.float32),
        np.asarray(inputs["Wq"], np.float32),
        np.asarray(inputs["bq"], np.float32),
        np.asarray(inputs["Wk"], np.float32),
        np.asarray(inputs["bk"], np.float32),
        np.asarray(inputs["Wv"], np.float32),
        np.asarray(inputs["bv"], np.float32),
    )
    res = bass_utils.run_bass_kernel_spmd(
        nc, in_maps, list(range(NCORES)), trace=trace
    )
    return _combine(res.results), res.exec_time_ns


def kernel(**inputs):
    out, _ = _run(trace=False, **inputs)
    return out


# revision 22
# speedup vs baseline: 1.1800x; 1.1800x over previous
"""Trainium2 Bass kernel: single-head causal attention.

B=4, T=4096, E=512, H=64, fp32 in/out.

Sharding: 2 cores per batch sample. Each core computes partial softmax
(numerator and denominator) for ALL 4096 queries of its sample over HALF
the keys: core 2b takes even 128-key-strips, core 2b+1 odd strips. The
host combines partials: out = (num0+num1)/(den0+den1).

Token rotation: the host rotates every 256-token block by 128*rho so
each core's keys are the first 128 tokens of every 256-block (identical
SPMD program on all cores). Host un-permutes output columns; the causal
masks carry the rotation.

Device kernel per core (bf16 matmul operands, fp32 PSUM accumulate):
  - ROW-TILED scores: the two strips of a pair run concurrently in the
    PE array (strip A rows 0:64, strip B rows 64:128 via operand base
    partitions). Q projection duplicated to both partition halves
    ([Wq|Wq] stationary). The kv projection is PARITY-SPLIT so odd
    strips' K^T lands directly at partitions 64:128: even-strip token
    columns use the [Wk|Wv] stationary, odd-strip columns [Wv|Wk]
    (so odd strips' V^T is at rows 0:64 instead).
  - V^T -> V (natural [k,h]) via PE transpose (4 per kv chunk into one
    bf16 PSUM tile) + one strided DVE copy per chunk; a ones column is
    packed after each V strip so the PV matmul (M=65) also produces the
    softmax denominator (partition-dim reduction on the PE).
  - exp on the scalar engine with fused 1/sqrt(H) scale; no max
    subtraction (scores bounded; fp32 exp cannot overflow here).
  - Diagonal pair of each chunk: the fully-masked half of the odd strip
    is skipped (scores/exp/PV restricted to query cols 256:512); the
    exp covers the contiguous [256:1024] window of the pair tile.
  - Causal mask applied multiplicatively after exp on the vector
    engine (256-col windows per diagonal strip).
  - Emission is software-pipelined: chunk c's first scores pair goes
    out before the c+1 projections, and scores/exp of pair j+1 precede
    PV of pair j, so the in-order PE never waits on exp or on input
    DMAs.
  - PE warm-up matmuls run during the input DMA window so real matmuls
    start at the full 2.4 GHz clock (HAM un-throttled).
"""

import functools

import numpy as np
import ml_dtypes

B, T, E, H = 4, 4096, 512, 64
NCORES = 8
NCHUNK = 8  # 512-query chunks per sample
CHUNK = T // NCHUNK  # 512
NSTRIP = 16  # local 128-key strips per core (half of T/128)
VSTRIDE = 80  # per-strip stride in the packed V tile

bf16 = ml_dtypes.bfloat16


@functools.lru_cache(maxsize=1)
def _build():
    import concourse.mybir as mybir
    from concourse import bacc
    from concourse.masks import make_identity
    import concourse.tile as tile

    dt_bf = mybir.dt.bfloat16
    dt_f32 = mybir.dt.float32

    nc = bacc.Bacc("TRN2", target_bir_lowering=False, num_devices=NCORES)

    # x^T, rotated, (quarter, e-strip)-blocked:
    # [4 quarters, 128, 4 e-strips, 1024 tokens]
    xt = nc.dram_tensor("xt", [4, 128, 4, T // 4], dt_bf, kind="ExternalInput")
    # [Wq | Wq] per e-strip (duplicated for row-tiled scores)
    wq = nc.dram_tensor("wq", [128, 4 * 128], dt_bf, kind="ExternalInput")
    # per e-strip: [Wk|Wv] (even-strip cols) then [Wv|Wk] (odd-strip cols)
    wkv = nc.dram_tensor("wkv", [128, 4 * 256], dt_bf, kind="ExternalInput")
    bias_q = nc.dram_tensor("bias_q", [128, 1], dt_f32, kind="ExternalInput")
    # [bk|bv] and [bv|bk]
    bias_kv = nc.dram_tensor("bias_kv", [128, 2], dt_f32, kind="ExternalInput")
    masks = nc.dram_tensor("masks", [128, CHUNK], dt_bf, kind="ExternalInput")
    out_d = nc.dram_tensor("out", [H + 1, T], dt_f32, kind="ExternalOutput")

    AF = mybir.ActivationFunctionType

    with tile.TileContext(nc) as tc:
        with (
            tc.tile_pool(name="const", bufs=1) as cpool,
            tc.tile_pool(name="xt_pool", bufs=1) as xpool,
            tc.tile_pool(name="q_pool", bufs=NCHUNK) as qpool,
            tc.tile_pool(name="kv_pool", bufs=4) as kvpool,
            tc.tile_pool(name="v_pool", bufs=1) as vpool,
            tc.tile_pool(name="p_pool", bufs=3) as ppool,
            tc.tile_pool(name="o_pool", bufs=2) as opool,
            tc.tile_pool(name="ps_proj", bufs=2, space="PSUM") as pspr_pool,
            tc.tile_pool(name="ps_s", bufs=2, space="PSUM") as pss_pool,
            tc.tile_pool(name="ps_o", bufs=2, space="PSUM") as pso_pool,
        ):
            # ---- input DMAs. The critical path (wkv + quarter-0
            # e-strips) leads on the sync queue, followed by the other
            # quarters in order; the scalar queue carries only the small
            # weight/bias/mask transfers so nothing competes with the
            # quarter-0 stream for SDMA bandwidth ----
            xt_sb = xpool.tile([128, 4 * T], dt_bf)
            wkv_sb = cpool.tile([128, 4 * 256], dt_bf)
            nc.sync.dma_start(wkv_sb, wkv.ap())
            for es in range(4):
                nc.sync.dma_start(
                    xt_sb[:, es * 1024 : (es + 1) * 1024], xt.ap()[0][:, es, :]
                )
            wq_sb = cpool.tile([128, 4 * 128], dt_bf)
            nc.scalar.dma_start(wq_sb, wq.ap())
            bkv_sb = cpool.tile([128, 2], dt_f32)
            nc.scalar.dma_start(bkv_sb, bias_kv.ap())
            bq_sb = cpool.tile([128, 1], dt_f32)
            nc.scalar.dma_start(bq_sb, bias_q.ap())
            masks_sb = cpool.tile([128, CHUNK], dt_bf)
            nc.scalar.dma_start(masks_sb, masks.ap())
            for qd in range(1, 4):
                nc.sync.dma_start(
                    xt_sb[:, qd * T : qd * T + T // 2],
                    xt.ap()[qd][:, 0:2, :].rearrange("p a t -> p (a t)"),
                )
                nc.sync.dma_start(
                    xt_sb[:, qd * T + T // 2 : (qd + 1) * T],
                    xt.ap()[qd][:, 2:4, :].rearrange("p a t -> p (a t)"),
                )
            ident = cpool.tile([128, 128], dt_bf)
            make_identity(nc, ident)
            junk_sb = cpool.tile([128, CHUNK], dt_bf)
            nc.vector.memset(junk_sb, 0.0)

            # ---- PE warm-up (results never read) ----
            for i in range(2):
                ps_w = pss_pool.tile([128, 2 * CHUNK], dt_f32, tag="pss")
                for r in range(2 if i == 0 else 1):
                    nc.tensor.matmul(
                        ps_w[:, r * CHUNK : (r + 1) * CHUNK],
                        lhsT=junk_sb[:, 0:128],
                        rhs=junk_sb,
                        start=True,
                        stop=True,
                        skip_group_check=True,
                    )

            # packed V (natural [k,h] layout + ones column for denominator)
            v_nat = vpool.tile([128, NSTRIP * VSTRIDE], dt_bf)
            v3 = v_nat.rearrange("p (s c) -> p s c", c=VSTRIDE)
            nc.vector.memset(v3[:, :, 64:65], 1.0)

            def xt_block(qd, es):
                off = (qd * 4 + es) * 1024
                return xt_sb[:, off : off + 1024]

            scale = 1.0 / float(np.sqrt(H))
            kv_tiles = []
            q_tiles = []

            def kv_proj(ckv):
                # kv tile layout: cols [0:256] = even strips (0,2) with
                # [K^T;V^T] rows, cols [256:512] = odd strips (1,3) with
                # [V^T;K^T] rows (so odd K^T sits at partitions 64:128).
                # The two parity accumulation chains must live in
                # DIFFERENT PSUM banks: a start=True matmul clears the
                # has_written bits of its whole bank, so a second chain's
                # start would break the first chain's accumulation.
                ps_e = pspr_pool.tile([128, 256], dt_f32, tag="proj")
                ps_od = pss_pool.tile([128, 2 * CHUNK], dt_f32, tag="pss")
                for es in range(4):
                    key_rhs = xt_block(ckv, es).rearrange(
                        "p (a two b) -> p a two b", two=2, b=128
                    )[:, :, 0, :]
                    nc.tensor.matmul(
                        ps_e,
                        lhsT=wkv_sb[:, es * 256 : es * 256 + 128],
                        rhs=key_rhs[:, 0::2, :],
                        start=(es == 0),
                        stop=(es == 3),
                    )
                    nc.tensor.matmul(
                        ps_od[:, 0:256],
                        lhsT=wkv_sb[:, es * 256 + 128 : es * 256 + 256],
                        rhs=key_rhs[:, 1::2, :],
                        start=(es == 0),
                        stop=(es == 3),
                    )
                kv_sb = kvpool.tile([128, CHUNK], dt_bf, tag="kv")
                nc.vector.tensor_scalar_add(
                    kv_sb[:, 0:256], ps_e, bkv_sb[:, 0:1]
                )
                nc.vector.tensor_scalar_add(
                    kv_sb[:, 256:512], ps_od[:, 0:256], bkv_sb[:, 1:2]
                )
                kv_tiles.append(kv_sb)

            def v_transpose(ckv):
                # V^T -> natural V strips via PE transpose. Even strips'
                # V^T is at rows 64:128 (-> V cols 64:128 of the
                # transpose), odd strips' at rows 0:64 (-> V cols 0:64).
                # Transposed block order: [s0, s2, s1, s3].
                kv_sb = kv_tiles[ckv]
                ps_tr = pspr_pool.tile([128, CHUNK], dt_bf, tag="proj")
                for j in range(4):
                    nc.tensor.transpose(
                        ps_tr[:, j * 128 : (j + 1) * 128],
                        kv_sb[:, j * 128 : (j + 1) * 128],
                        ident,
                    )
                sl = v3[:, 4 * ckv : 4 * ckv + 4, 0:64]
                ev = ps_tr[:, 64:320].rearrange("p (s c) -> p s c", c=128)
                od = ps_tr[:, 256:512].rearrange("p (s c) -> p s c", c=128)
                # even strips s0, s2 -> v slots 0, 2; odd s1, s3 -> 1, 3
                nc.vector.tensor_copy(sl[:, 0::2, :], ev[:, :, 0:64])
                nc.vector.tensor_copy(sl[:, 1::2, :], od[:, :, 0:64])

            def q_proj(c):
                ps_q = pspr_pool.tile([128, CHUNK], dt_f32, tag="proj")
                for es in range(4):
                    nc.tensor.matmul(
                        ps_q,
                        lhsT=wq_sb[:, es * 128 : (es + 1) * 128],
                        rhs=xt_block(c // 2, es)[
                            :, (c % 2) * CHUNK : (c % 2) * CHUNK + CHUNK
                        ],
                        start=(es == 0),
                        stop=(es == 3),
                    )
                q_sb = qpool.tile([128, CHUNK], dt_bf, tag="q")
                nc.vector.tensor_scalar_add(q_sb, ps_q, bq_sb)
                q_tiles.append(q_sb)

            def k_lo(l):
                # even strip l: K^T at partitions 0:64, even half cols
                o = ((l % 4) // 2) * 128
                return kv_tiles[l // 4][0:64, o : o + 128]

            def k_hi(l):
                # odd strip l: K^T at partitions 64:128, odd half cols
                o = 256 + ((l % 4) // 2) * 128
                return kv_tiles[l // 4][64:128, o : o + 128]

            kv_proj(0)
            q_proj(0)
            for c in range(NCHUNK):
                ns = 2 * (c + 1)
                q_sb = q_tiles[c]
                ps_o = pso_pool.tile([H + 1, CHUNK], dt_f32, tag="pso")

                def scores_exp(j):
                    le, lo = 2 * j, 2 * j + 1
                    diag = j == c
                    ps_s = pss_pool.tile([128, 2 * CHUNK], dt_f32, tag="pss")
                    p_sb = ppool.tile([128, 2 * CHUNK], dt_bf, tag="p")
                    if not diag:
                        nc.tensor.matmul(
                            ps_s[:, 0:CHUNK],
                            lhsT=k_lo(le),
                            rhs=q_sb[0:64, :],
                            start=True,
                            stop=True,
                        )
                        nc.tensor.matmul(
                            ps_s[:, CHUNK : 2 * CHUNK],
                            lhsT=k_hi(lo),
                            rhs=q_sb[64:128, :],
                            start=True,
                            stop=True,
                        )
                        nc.scalar.activation(p_sb, ps_s, AF.Exp, scale=scale)
                    else:
                        # diagonal pair: odd strip only needs query cols
                        # 256:512; layout [odd | even] so one exp covers
                        # the contiguous [256:1024] window
                        nc.tensor.matmul(
                            ps_s[:, 256:CHUNK],
                            lhsT=k_hi(lo),
                            rhs=q_sb[64:128, 256:CHUNK],
                            start=True,
                            stop=True,
                        )
                        nc.tensor.matmul(
                            ps_s[:, CHUNK : 2 * CHUNK],
                            lhsT=k_lo(le),
                            rhs=q_sb[0:64, :],
                            start=True,
                            stop=True,
                        )
                        nc.scalar.activation(
                            p_sb[:, 256 : 2 * CHUNK],
                            ps_s[:, 256 : 2 * CHUNK],
                            AF.Exp,
                            scale=scale,
                        )
                        nc.vector.tensor_mul(
                            p_sb[:, 256:CHUNK],
                            p_sb[:, 256:CHUNK],
                            masks_sb[:, 0:256],
                        )
                        nc.vector.tensor_mul(
                            p_sb[:, CHUNK : CHUNK + 256],
                            p_sb[:, CHUNK : CHUNK + 256],
                            masks_sb[:, 256:512],
                        )
                    return p_sb

                def pv(j, p_sb):
                    le, lo = 2 * j, 2 * j + 1
                    diag = j == c
                    if not diag:
                        nc.tensor.matmul(
                            ps_o,
                            lhsT=v_nat[:, le * VSTRIDE : le * VSTRIDE + 65],
                            rhs=p_sb[:, 0:CHUNK],
                            start=(le == 0),
                            stop=False,
                        )
                        nc.tensor.matmul(
                            ps_o,
                            lhsT=v_nat[:, lo * VSTRIDE : lo * VSTRIDE + 65],
                            rhs=p_sb[:, CHUNK : 2 * CHUNK],
                            start=False,
                            stop=(lo == ns - 1),
                        )
                    else:
                        nc.tensor.matmul(
                            ps_o,
                            lhsT=v_nat[:, le * VSTRIDE : le * VSTRIDE + 65],
                            rhs=p_sb[:, CHUNK : 2 * CHUNK],
                            start=(le == 0),
                            stop=False,
                        )
                        nc.tensor.matmul(
                            ps_o[:, 256:CHUNK],
                            lhsT=v_nat[:, lo * VSTRIDE : lo * VSTRIDE + 65],
                            rhs=p_sb[:, 256:CHUNK],
                            start=False,
                            stop=(lo == ns - 1),
                            skip_group_check=True,
                        )

                # chunk c's first scores pair goes out BEFORE the c+1
                # projections (which may wait on later input DMAs)
                prev = scores_exp(0)
                if c % 2 == 0:
                    v_transpose(c // 2)
                if c + 1 < NCHUNK:
                    if (c + 1) % 2 == 0:
                        kv_proj((c + 1) // 2)
                    q_proj(c + 1)
                for j in range(1, c + 1):
                    cur = scores_exp(j)
                    pv(j - 1, prev)
                    prev = cur
                pv(c, prev)

                o_sb = opool.tile([H + 1, CHUNK], dt_f32, tag="o")
                nc.vector.tensor_copy(o_sb, ps_o)
                nc.sync.dma_start(
                    out_d.ap()[:, c * CHUNK : (c + 1) * CHUNK], o_sb
                )

    nc.compile()
    return nc


def _perm(rho):
    """Rotated-order permutation: rotated position i holds original token
    perm[i]. Involutive (half swap within each 256-block)."""
    i = np.arange(T)
    return (i // 256) * 256 + ((i % 256) + 128 * rho) % 256


def _make_in_maps(x, Wq, bq, Wk, bk, Wv, bv):
    wq_r = Wq.reshape(4, 128, 64)
    wq_pack = np.ascontiguousarray(
        np.concatenate([wq_r, wq_r], axis=2).transpose(1, 0, 2).reshape(128, 512)
    ).astype(bf16)
    wk_r = Wk.reshape(4, 128, 64)
    wv_r = Wv.reshape(4, 128, 64)
    # per e-strip: [Wk|Wv] then [Wv|Wk]
    wkv_pack = np.ascontiguousarray(
        np.concatenate([wk_r, wv_r, wv_r, wk_r], axis=2)
        .transpose(1, 0, 2)
        .reshape(128, 1024)
    ).astype(bf16)
    bias_q = np.ascontiguousarray(
        np.concatenate([bq, bq])[:, None]
    ).astype(np.float32)
    bias_kv = np.ascontiguousarray(
        np.stack([np.concatenate([bk, bv]), np.concatenate([bv, bk])], axis=1)
    ).astype(np.float32)

    kk = np.arange(128)[:, None]
    in_maps = []
    for b in range(B):
        xt_b = np.ascontiguousarray(x[b].T).astype(bf16).reshape(4, 128, T)
        for rho in range(2):
            perm = _perm(rho)
            xt_rot = xt_b[:, :, perm]  # rotated token order
            xt_in = np.ascontiguousarray(
                xt_rot.reshape(4, 128, 4, T // 4).transpose(2, 1, 0, 3)
            )
            v = perm[:CHUNK]
            m0 = (kk - v[None, :] <= -128 * rho).astype(bf16)
            m1 = (kk - v[None, :] <= -256 - 128 * rho).astype(bf16)
            # [odd-strip window cols 256:512 | even-strip cols 0:256]
            masks_np = np.ascontiguousarray(
                np.concatenate([m1[:, 256:512], m0[:, 0:256]], axis=1)
            )
            in_maps.append(
                {
                    "xt": xt_in,
                    "wq": wq_pack,
                    "wkv": wkv_pack,
                    "bias_q": bias_q,
                    "bias_kv": bias_kv,
                    "masks": masks_np,
                }
            )
    return in_maps


def _combine(results):
    out = np.empty((B, T, H), np.float32)
    p1 = _perm(1)
    for b in range(B):
        a0 = results[2 * b]["out"].astype(np.float64)
        a1 = results[2 * b + 1]["out"].astype(np.float64)
        a1 = a1[:, p1]  # un-rotate core-1 columns (involutive perm)
        num = a0[:H] + a1[:H]
        den = a0[H] + a1[H]
        out[b] = (num / den).T.astype(np.float32)
    return out


def _run(trace=False, **inputs):
    from concourse import bass_utils

    nc = _build()
    in_maps = _make_in_maps(
        np.asarray(inputs["x"], np.float32),
        np.asarray(inputs["Wq"], np.float32),
        np.asarray(inputs["bq"], np.float32),
        np.asarray(inputs["Wk"], np.float32),
        np.asarray(inputs["bk"], np.float32),
        np.asarray(inputs["Wv"], np.float32),
        np.asarray(inputs["bv"], np.float32),
    )
    res = bass_utils.run_bass_kernel_spmd(
        nc, in_maps, list(range(NCORES)), trace=trace
    )
    return _combine(res.results), res.exec_time_ns


def kernel(**inputs):
    out, _ = _run(trace=False, **inputs)
    return out
